# revision 26
# baseline (speedup 1.0000x reference)
"""Trainium2 Bass kernel for the few-shot knn-attention module.

Pipeline per sample (512 ch, 16x16 spatial):
  mask = softmax_{c,h,w}(W @ x + b); xm = x * mask  (mask unnormalized on
  device; the softmax denominator and exp(b) are folded algebraically)
  prototypes s = mean over 5 shots+space of xm; queries scored by cosine
  similarity against s; softmax over classes; mean over space -> (75, 5).

Distribution: data-parallel over the 100 samples on 8 NeuronCores
(14 slots/core = 4 shot + 10 query slots, zero-padded).  Prototype partial
sums are exchanged with an AllGather (cheaper than AllReduce) and summed
locally, overlapped with query processing: the shot slots are processed
first and the collective is issued before any query-side DMA is enqueued,
so the 10KB exchange never queues behind bulk input traffic.

Implementation notes:
  - 1x1 conv optionally runs in fp8(e4m3) DoubleRow mode (weights
    pre-scaled x16, the exp() descales by 1/16), halving PE time vs bf16.
  - score/norm reductions use "transposed" matmuls: per-position values
    land with positions on PSUM partitions and tiny (<=5 col) outputs, so
    the class softmax runs fully batched across all 10 queries.
  - the class-norm factor (10/||s||) is applied on the logits tensor, not
    the prototypes, keeping it off the collective->scores critical path.
  - a short burst of dummy matmuls at t=0 keeps the PE pstate ramp warm
    while the first input DMAs land.
"""

import numpy as np
import ml_dtypes

import concourse.bass as bass
import concourse.mybir as mybir
import concourse.tile as tile
from concourse import bacc
from concourse.bass_utils import run_bass_kernel_spmd

# Force the act-table chooser onto the one set containing BOTH Exp and Ln
# ("natural_log_exp_and_others") so the kernel pays a single table load
# instead of swapping between the exp-only and ln-only sets (~2.7us each).
import concourse.hw_specs as _hw_specs

_ORIG_GET_ACT_TABLES = _hw_specs.get_activation_tables


def _nl_exp_only_tables(arch):
    t = _ORIG_GET_ACT_TABLES(arch)
    return {
        k: (v if k == "natural_log_exp_and_others" else set()) for k, v in t.items()
    }


bacc.get_activation_tables = _nl_exp_only_tables

N_CORES = 8
WAY = 5
SHOT = 5
C = 512
F = 256  # 16*16
KO = C // 128  # 4 partition tiles of the channel dim
NSHOT_SLOTS = 4
NQ_SLOTS = 10
NSLOTS = NSHOT_SLOTS + NQ_SLOTS
SHOTS_PER_CORE = [4, 3, 3, 3, 3, 3, 3, 3]       # sums to 25
QUERIES_PER_CORE = [9, 10, 10, 10, 9, 9, 9, 9]  # sums to 75
LN10 = float(np.log(10.0))
CONV_FP8 = True
W_SCALE = 16.0  # fp8 weight pre-scale; exp() descales via scale=1/W_SCALE
N_WARM = 11     # dummy matmuls bridging the PE pstate ramp at startup

F32 = mybir.dt.float32
BF16 = mybir.dt.bfloat16
FP8 = mybir.dt.float8e4
EXP = mybir.ActivationFunctionType.Exp
LN = mybir.ActivationFunctionType.Ln
MULT = mybir.AluOpType.mult
ADD = mybir.AluOpType.add
DR = mybir.MatmulPerfMode.DoubleRow


def build_nc(skip_collective=False, conv_fp8=CONV_FP8):
    nc = bacc.Bacc(None, target_bir_lowering=False)
    if conv_fp8:
        xs8 = nc.dram_tensor("xs8", [NSLOTS, 128, KO * F], FP8, kind="ExternalInput")
        wt8 = nc.dram_tensor("wt8", [128, KO * C], FP8, kind="ExternalInput")
    xbf = nc.dram_tensor("xbf", [NSLOTS, 128, KO * F], BF16, kind="ExternalInput")
    if not conv_fp8:
        wtb = nc.dram_tensor("wtb", [128, KO * C], BF16, kind="ExternalInput")
    # bias (exp'd per-channel) and the per-shot one-hot class rows, one DMA
    bsw = nc.dram_tensor(
        "bsw", [128, KO + NSHOT_SLOTS * WAY], F32, kind="ExternalInput"
    )
    out = nc.dram_tensor("out", [1, NQ_SLOTS * WAY], F32, kind="ExternalOutput")

    with tile.TileContext(nc) as tc:
        with (
            tc.tile_pool(name="singles", bufs=1) as singles,
            tc.tile_pool(name="xepool", bufs=3) as xepool,
            tc.tile_pool(name="xms", bufs=2) as xms_pool,
            tc.tile_pool(name="xmq", bufs=NQ_SLOTS) as xmq_pool,
            tc.tile_pool(name="xm2", bufs=3) as xm2_pool,
            tc.tile_pool(name="work", bufs=8) as work,
            tc.tile_pool(name="pconv", bufs=2, space="PSUM") as pconv,
            tc.tile_pool(name="pscratch", bufs=1, space="PSUM") as pscratch,
            tc.tile_pool(name="psmall", bufs=1, space="PSUM") as psmall,
            tc.tile_pool(name="dram", bufs=2, space="DRAM") as dram,
        ):
            # ---------------- constants ----------------
            warm_bf = singles.tile([128, 256], BF16)
            nc.vector.memset(warm_bf, 0.0)
            onesC_f32 = singles.tile([128, 128], F32)
            nc.vector.memset(onesC_f32, 1.0)
            onesC_bf = singles.tile([128, 128], BF16)
            nc.vector.memset(onesC_bf, 1.0)
            onesF_bf = singles.tile([128, 1], BF16)
            nc.vector.memset(onesF_bf, 1.0 / F)
            c_eps = singles.tile([128, 1], F32)
            nc.vector.memset(c_eps, 1e-30)
            c_ln10 = singles.tile([128, 1], F32)
            nc.vector.memset(c_ln10, LN10)

            # ---- early DMA group: bias+sw, shot inputs, weights ----
            bs_sb = singles.tile([128, KO + NSHOT_SLOTS * WAY], F32)
            nc.sync.dma_start(bs_sb, bsw[:])
            bias_sb = bs_sb[:, 0:KO]
            sw_sb = bs_sb[:, KO:].rearrange("p (s m) -> p s m", s=NSHOT_SLOTS)
            xball = singles.tile([128, NSLOTS, KO, F], BF16)
            xbr = xbf.rearrange("s p (ko f) -> p s ko f", ko=KO)
            if conv_fp8:
                wt_sb = singles.tile([128, KO, C], FP8)
                x8all = singles.tile([128, NSLOTS, KO, F], FP8)
                xsr = xs8.rearrange("s p (ko f) -> p s ko f", ko=KO)
                nc.sync.dma_start(x8all[:, 0:2], xsr[:, 0:2])
                nc.sync.dma_start(wt_sb, wt8.rearrange("p (ko o) -> p ko o", ko=KO))
                # bf16 copies for the shot mask-multiply (2x DVE mode)
                nc.sync.dma_start(xball[:, 0:2], xbr[:, 0:2])
                nc.sync.dma_start(x8all[:, 2:4], xsr[:, 2:4])
                nc.sync.dma_start(xball[:, 2:4], xbr[:, 2:4])
            else:
                wt_sb = singles.tile([128, KO, C], BF16)
                nc.sync.dma_start(xball[:, 0:2], xbr[:, 0:2])
                nc.sync.dma_start(wt_sb, wtb.rearrange("p (ko o) -> p ko o", ko=KO))
                nc.sync.dma_start(xball[:, 2:4], xbr[:, 2:4])

            # eb[c] = exp(b[c]); the conv bias is applied multiplicatively and
            # folded into prototypes, query norms and the score weights.
            eb = singles.tile([128, KO], F32)
            nc.scalar.activation(eb, bias_sb, EXP)
            eb_bf = singles.tile([128, KO, 1], BF16)
            nc.vector.tensor_copy(eb_bf, eb[:, :, None])
            ebsq = singles.tile([128, KO], F32)
            nc.vector.tensor_mul(ebsq, eb, eb)
            eb2_bf = singles.tile([128, KO, 1], BF16)
            nc.vector.tensor_copy(eb2_bf, ebsq[:, :, None])

            red4 = singles.tile([128, NSHOT_SLOTS, KO, 1], F32)
            qeS = singles.tile([128, NSHOT_SLOTS], BF16)
            pall = singles.tile([128, NQ_SLOTS, 2, WAY], BF16)
            out_sb = singles.tile([1, NQ_SLOTS * WAY], F32)

            # score/norm PSUM: [query, f-half, 5 scores + |q|^2] for all 10
            # queries plus the shot column-sum scratch, all in one bank.
            psall = psmall.tile([128, 512], F32)
            psT = psall[:, 0 : NQ_SLOTS * 2 * (WAY + 1)].rearrange(
                "p (a h m) -> p a h m", a=NQ_SLOTS, h=2
            )
            qe_ps = psall[:, 128 : 128 + NSHOT_SLOTS]
            ps_s2 = psall[:, 136 : 136 + NSHOT_SLOTS]
            ps_sn = psall[:, 144 : 144 + WAY]

            # ---------------- PE warmup (bridges the pstate ramp) ----------
            for w in range(N_WARM):
                ps_w = pscratch.tile([128, 256], F32, tag="warm", name=f"warm{w}")
                nc.tensor.matmul(ps_w, warm_bf[:, :128], warm_bf, start=True, stop=True)

            xe_tiles = [None] * NSLOTS
            xm_tiles = [None] * NQ_SLOTS
            xm2_tiles = [None] * NQ_SLOTS

            def conv_slot(i):
                """1x1 conv for slot i -> psum tile, then exp -> xe (SBUF)."""
                ps = pconv.tile([128, KO, F], F32, tag="conv", name=f"conv{i}")
                if conv_fp8:
                    for oo in range(KO):
                        for k2 in range(KO // 2):
                            nc.tensor.matmul(
                                ps[:, oo, :],
                                wt_sb[
                                    :, 2 * k2 : 2 * k2 + 2, 128 * oo : 128 * (oo + 1)
                                ],
                                x8all[:, i, 2 * k2 : 2 * k2 + 2, :],
                                start=(k2 == 0),
                                stop=(k2 == KO // 2 - 1),
                                perf_mode=DR,
                            )
                else:
                    for oo in range(KO):
                        for k in range(KO):
                            nc.tensor.matmul(
                                ps[:, oo, :],
                                wt_sb[:, k, 128 * oo : 128 * (oo + 1)],
                                xball[:, i, k, :],
                                start=(k == 0),
                                stop=(k == KO - 1),
                            )
                xe = xepool.tile([128, KO, F], BF16, tag="xe")
                xe_tiles[i] = xe
                scale = (1.0 / W_SCALE) if conv_fp8 else 1.0
                nc.scalar.activation(xe, ps, EXP, scale=scale)
                return xe

            def qe_mms(i):
                # column sums of eb*xe land on f-partitions; both f-halves
                # accumulate into one column so S = colsum(qe) directly.
                xe = xe_tiles[i]
                for h in range(2):
                    for k in range(KO):
                        nc.tensor.matmul(
                            qe_ps[:, i : i + 1],
                            xe[:, k, 128 * h : 128 * (h + 1)],
                            eb_bf[:, k, :],
                            start=(h == 0 and k == 0),
                            stop=(h == 1 and k == KO - 1),
                        )

            def norm_mms(j):
                # |q|^2 per position via transposed matmuls into psT col 5
                xm2 = xm2_tiles[j]
                for h in range(2):
                    for k in range(KO):
                        nc.tensor.matmul(
                            psT[:, j, h, WAY : WAY + 1],
                            xm2[:, k, 128 * h : 128 * (h + 1)],
                            eb2_bf[:, k, :],
                            start=(k == 0),
                            stop=(k == KO - 1),
                        )

            # ---------------- shot slots ----------------
            for i in range(NSHOT_SLOTS):
                xe = conv_slot(i)
                xms = xms_pool.tile([128, KO, F], BF16, tag="xms")
                xf = xms_pool.tile([128, KO, F // 2], BF16, tag="xf")
                xmask = xball[:, i]
                with tc.high_priority():
                    nc.vector.tensor_mul(xms, xmask, xe)
                    # fold halves at 2x speed, then a half-length 1x reduce
                    nc.vector.tensor_tensor(
                        xf, xms[:, :, 0 : F // 2], xms[:, :, F // 2 : F], ADD
                    )
                    nc.vector.reduce_sum(red4[:, i], xf, axis=mybir.AxisListType.X)
                if i > 0:
                    qe_mms(i - 1)
            qe_mms(NSHOT_SLOTS - 1)

            # ---------------- shot epilogue -> AllGather ----------------
            with tc.high_priority():
                # softmax denominators: S[i] = colsum(qe), then per-shot
                # weights ebw5[k,m,i] = eb[k] * sw[i,m] / S[i] (off the
                # masked-sum critical path; consumed once red4h is complete)
                nc.vector.tensor_copy(qeS, qe_ps)
                nc.tensor.matmul(ps_s2, onesC_bf, qeS, start=True, stop=True)
                rS4 = work.tile([128, NSHOT_SLOTS], F32, tag="rS4")
                nc.vector.reciprocal(rS4, ps_s2)
                w5all = work.tile([128, NSHOT_SLOTS, WAY], F32, tag="w5all")
                nc.vector.tensor_tensor(
                    w5all,
                    sw_sb,
                    rS4[:, :, None].to_broadcast([128, NSHOT_SLOTS, WAY]),
                    MULT,
                )
                ebw5 = work.tile([128, KO, WAY, NSHOT_SLOTS], F32, tag="ebw5")
                nc.vector.tensor_tensor(
                    ebw5,
                    w5all.rearrange("p i m -> p m i")[:, None, :, :].to_broadcast(
                        [128, KO, WAY, NSHOT_SLOTS]
                    ),
                    eb[:, :, None, None].to_broadcast([128, KO, WAY, NSHOT_SLOTS]),
                    MULT,
                )
                contrib = work.tile([128, KO, WAY, NSHOT_SLOTS], F32, tag="contrib")
                nc.vector.tensor_tensor(
                    contrib,
                    red4.rearrange("p i k e -> p k i e")[:, :, None, :, 0].to_broadcast(
                        [128, KO, WAY, NSHOT_SLOTS]
                    ),
                    ebw5,
                    MULT,
                )
                proto = work.tile([128, KO, WAY, 1], F32, tag="proto")
                nc.vector.reduce_sum(proto, contrib, axis=mybir.AxisListType.X)

                ar_in = dram.tile([128, KO, WAY], F32, tag="ar_in")
                ar_out = dram.tile([N_CORES, 128, KO, WAY], F32, tag="ar_out")
                nc.sync.dma_start(ar_in, proto[:, :, :, 0])
            if skip_collective:
                for r in range(N_CORES):
                    nc.gpsimd.dma_start(ar_out[r], ar_in[:])
            else:
                nc.gpsimd.collective_compute(
                    "AllGather",
                    mybir.AluOpType.bypass,
                    replica_groups=[list(range(N_CORES))],
                    ins=[ar_in[:].opt()],
                    outs=[ar_out[:].opt()],
                )

            # ---- late DMA group: query inputs. Per-slot chunks keep the
            # DMA-engine FIFO shallow (the 625ns/issue HWDGE throttle paces
            # arrivals at the transfer rate), so the 10KB ar_in write slots
            # in with at most one chunk of queue delay. ----
            for i in range(NSHOT_SLOTS, NSLOTS):
                if conv_fp8:
                    nc.sync.dma_start(x8all[:, i : i + 1], xsr[:, i : i + 1])
            for i in range(NSHOT_SLOTS, NSLOTS):
                nc.sync.dma_start(xball[:, i : i + 1], xbr[:, i : i + 1])

            # ---------------- query slots ----------------
            for j in range(NQ_SLOTS):
                i = NSHOT_SLOTS + j
                xe = conv_slot(i)
                xm = xmq_pool.tile([128, KO, F], BF16, tag="xmq")
                xm_tiles[j] = xm
                nc.vector.tensor_mul(xm, xball[:, i], xe)
                xm2 = xm2_pool.tile([128, KO, F], BF16, tag="xm2")
                xm2_tiles[j] = xm2
                nc.vector.tensor_mul(xm2, xm, xm)
                if j > 0:
                    norm_mms(j - 1)
            norm_mms(NQ_SLOTS - 1)

            # per-position 10/|q| for all queries (collective-independent)
            lnq = work.tile([128, NQ_SLOTS, 2], F32, tag="lnq")
            nc.scalar.activation(lnq, psT[:, :, :, WAY], LN, bias=c_eps)
            rq = work.tile([128, NQ_SLOTS, 2], F32, tag="rq")
            nc.scalar.activation(rq, lnq, EXP, bias=c_ln10, scale=-0.5)

            # ---------------- consume AllGather ----------------
            with tc.high_priority():
                protoAll = work.tile([128, N_CORES, KO, WAY], F32, tag="protoAll")
                nc.sync.dma_start(protoAll, ar_out.rearrange("r p k m -> p r k m"))
                protoG = work.tile([128, KO, WAY, 1], F32, tag="protoG")
                nc.vector.reduce_sum(
                    protoG,
                    protoAll.rearrange("p r k m -> p k m r"),
                    axis=mybir.AxisListType.X,
                )
                # unnormalized score weights: protoG*eb (the 10/||s|| factor
                # is folded into the logits via rqrsn below, off this path)
                s_hatU = work.tile([128, KO, WAY], BF16, tag="s_hatU")
                nc.vector.tensor_tensor(
                    s_hatU,
                    protoG[:, :, :, 0],
                    eb[:, :, None].to_broadcast([128, KO, WAY]),
                    MULT,
                )
                # ---- scores (transposed matmuls) ----
                for j in range(NQ_SLOTS):
                    xm = xm_tiles[j]
                    for h in range(2):
                        for k in range(KO):
                            nc.tensor.matmul(
                                psT[:, j, h, 0:WAY],
                                xm[:, k, 128 * h : 128 * (h + 1)],
                                s_hatU[:, k, :],
                                start=(k == 0),
                                stop=(k == KO - 1),
                            )
                # ---- parallel branch: rqrsn = rq[f] * 10/||protoG||[m] ----
                protosq = work.tile([128, KO, WAY], F32, tag="protosq")
                nc.vector.tensor_mul(protosq, protoG[:, :, :, 0], protoG[:, :, :, 0])
                for k in range(KO):
                    nc.tensor.matmul(
                        ps_sn,
                        onesC_f32,
                        protosq[:, k, :],
                        start=(k == 0),
                        stop=(k == KO - 1),
                    )
                snln = work.tile([128, WAY], F32, tag="snln")
                nc.scalar.activation(snln, ps_sn, LN, bias=c_eps)
                rsnb = work.tile([128, WAY], F32, tag="rsnb")
                nc.scalar.activation(rsnb, snln, EXP, scale=-0.5)
                rqrsn = work.tile([128, NQ_SLOTS, 2, WAY], F32, tag="rqrsn")
                nc.vector.tensor_tensor(
                    rqrsn,
                    rq[:, :, :, None].to_broadcast([128, NQ_SLOTS, 2, WAY]),
                    rsnb[:, None, None, :].to_broadcast([128, NQ_SLOTS, 2, WAY]),
                    MULT,
                )

                # ---------------- batched class softmax + spatial mean -----
                L = work.tile([128, NQ_SLOTS, 2, WAY], BF16, tag="L")
                nc.vector.tensor_tensor(L, psT[:, :, :, 0:WAY], rqrsn, MULT)
                E = work.tile([128, NQ_SLOTS, 2, WAY], BF16, tag="E")
                nc.scalar.activation(E, L, EXP)
                D = work.tile([128, NQ_SLOTS, 2, 1], F32, tag="D")
                nc.vector.reduce_sum(D, E, axis=mybir.AxisListType.X)
                R = work.tile([128, NQ_SLOTS, 2, 1], F32, tag="R")
                nc.vector.reciprocal(R, D)
                nc.vector.tensor_tensor(
                    pall, E, R.to_broadcast([128, NQ_SLOTS, 2, WAY]), MULT
                )
                psO = pscratch.tile([128, 256], F32, tag="warm", name="psO")
                for h in range(2):
                    nc.tensor.matmul(
                        psO[:1, : NQ_SLOTS * WAY],
                        onesF_bf,
                        pall[:, :, h, :],
                        start=(h == 0),
                        stop=(h == 1),
                    )
                nc.vector.tensor_copy(out_sb, psO[:1, : NQ_SLOTS * WAY])
                nc.sync.dma_start(out[:], out_sb[0:1, :])

    nc.finalize()
    return nc


_NC_CACHE = {}


def _get_nc():
    if "nc" not in _NC_CACHE:
        _NC_CACHE["nc"] = build_nc()
    return _NC_CACHE["nc"]


def _assignments():
    """Per-core (shot global ids, query global ids)."""
    shots = [20 * c + j for c in range(WAY) for j in range(SHOT)]
    queries = [20 * c + SHOT + j for c in range(WAY) for j in range(15)]
    so = np.cumsum([0] + SHOTS_PER_CORE)
    qo = np.cumsum([0] + QUERIES_PER_CORE)
    return [
        (shots[so[k] : so[k + 1]], queries[qo[k] : qo[k + 1]]) for k in range(N_CORES)
    ]


def _pack_slots(x_np, dtype):
    """[n, C, F] -> [n, 128, KO*F] p-major layout (1KB+ contiguous runs)."""
    n = x_np.shape[0]
    v = x_np.reshape(n, KO, 128, F).transpose(0, 2, 1, 3).reshape(n, 128, KO * F)
    return np.ascontiguousarray(v).astype(dtype)


def _pack_wt(W, dtype, scale=1.0):
    w = np.ascontiguousarray(W.T * scale)
    return np.ascontiguousarray(
        w.reshape(KO, 128, C).transpose(1, 0, 2).reshape(128, KO * C)
    ).astype(dtype)


def _make_in_maps(x, W, b):
    assign = _assignments()
    in_maps = []
    for k in range(N_CORES):
        s_list, q_list = assign[k]
        xs_core = np.zeros((NSLOTS, C, F), dtype=np.float32)
        xs_core[: len(s_list)] = x[s_list]
        xs_core[NSHOT_SLOTS : NSHOT_SLOTS + len(q_list)] = x[q_list]
        sw_core = np.zeros((NSHOT_SLOTS, WAY), dtype=np.float32)
        for slot, g in enumerate(s_list):
            sw_core[slot, g // 20] = 1.0
        bias_p = np.ascontiguousarray(b.reshape(KO, 128).T)  # [128, KO]
        bsw = np.concatenate(
            [
                bias_p,
                np.broadcast_to(
                    sw_core.reshape(1, NSHOT_SLOTS * WAY), (128, NSHOT_SLOTS * WAY)
                ),
            ],
            axis=1,
        ).astype(np.float32)
        m = {
            "xbf": _pack_slots(xs_core, ml_dtypes.bfloat16),
            "bsw": np.ascontiguousarray(bsw),
        }
        if CONV_FP8:
            m["xs8"] = _pack_slots(xs_core, ml_dtypes.float8_e4m3)
            m["wt8"] = _pack_wt(W, ml_dtypes.float8_e4m3, W_SCALE)
        else:
            m["wtb"] = _pack_wt(W, ml_dtypes.bfloat16)
        in_maps.append(m)
    return in_maps


def kernel(x, W, b):
    x = np.asarray(x, dtype=np.float32).reshape(100, C, F)
    W = np.asarray(W, dtype=np.float32)
    b = np.asarray(b, dtype=np.float32)

    nc = _get_nc()
    in_maps = _make_in_maps(x, W, b)
    res = run_bass_kernel_spmd(nc, in_maps, core_ids=list(range(N_CORES)))

    assign = _assignments()
    final = np.zeros((75, WAY), dtype=np.float32)
    for k in range(N_CORES):
        out_core = np.asarray(res.results[k]["out"], dtype=np.float32).reshape(
            NQ_SLOTS, WAY
        )
        _, q_list = assign[k]
        for slot, g in enumerate(q_list):
            c, j = divmod(g, 20)
            final[15 * c + (j - SHOT)] = out_core[slot]
    return final


# revision 53
# speedup vs baseline: 1.0697x; 1.0697x over previous
"""Trainium2 Bass kernel for the few-shot knn-attention module.

Pipeline per sample (512 ch, 16x16 spatial):
  mask = softmax_{c,h,w}(W @ x + b); xm = x * mask  (mask unnormalized on
  device; the softmax denominator and exp(b) are folded algebraically)
  prototypes s = mean over 5 shots+space of xm; queries scored by cosine
  similarity against s; softmax over classes; mean over space -> (75, 5).

Distribution: data-parallel over the 100 samples on 8 NeuronCores
(14 slots/core = 4 shot + 10 query slots, zero-padded).  Prototype partial
sums are exchanged with an AllGather (cheaper than AllReduce) and summed
locally, overlapped with query processing: the shot slots are processed
first and the collective is issued before any query-side DMA is enqueued,
so the 10KB exchange never queues behind bulk input traffic.

Implementation notes:
  - 1x1 conv optionally runs in fp8(e4m3) DoubleRow mode (weights
    pre-scaled x16, the exp() descales by 1/16), halving PE time vs bf16.
  - score/norm reductions use "transposed" matmuls: per-position values
    land with positions on PSUM partitions and tiny (<=5 col) outputs, so
    the class softmax runs fully batched across all 10 queries.
  - the class-norm factor (10/||s||) is applied on the logits tensor, not
    the prototypes, keeping it off the collective->scores critical path.
  - a short burst of dummy matmuls at t=0 keeps the PE pstate ramp warm
    while the first input DMAs land.
"""

import numpy as np
import ml_dtypes

import concourse.bass as bass
import concourse.mybir as mybir
import concourse.tile as tile
from concourse import bacc
from concourse.bass_utils import run_bass_kernel_spmd

# Force the act-table chooser onto the one set containing BOTH Exp and Ln
# ("natural_log_exp_and_others") so the kernel pays a single table load
# instead of swapping between the exp-only and ln-only sets (~2.7us each).
import concourse.hw_specs as _hw_specs

_ORIG_GET_ACT_TABLES = _hw_specs.get_activation_tables


def _nl_exp_only_tables(arch):
    t = _ORIG_GET_ACT_TABLES(arch)
    return {
        k: (v if k == "natural_log_exp_and_others" else set()) for k, v in t.items()
    }


bacc.get_activation_tables = _nl_exp_only_tables

N_CORES = 8
WAY = 5
SHOT = 5
C = 512
F = 256  # 16*16
KO = C // 128  # 4 partition tiles of the channel dim
NSHOT_SLOTS = 4
NQ_SLOTS = 10
NSLOTS = NSHOT_SLOTS + NQ_SLOTS
SHOTS_PER_CORE = [4, 3, 3, 3, 3, 3, 3, 3]       # sums to 25
QUERIES_PER_CORE = [9, 10, 10, 10, 9, 9, 9, 9]  # sums to 75
LN10 = float(np.log(10.0))
CONV_FP8 = True
W_SCALE = 16.0  # fp8 weight pre-scale; exp() descales via scale=1/W_SCALE
N_WARM = 11     # dummy matmuls bridging the PE pstate ramp at startup

F32 = mybir.dt.float32
BF16 = mybir.dt.bfloat16
FP8 = mybir.dt.float8e4
EXP = mybir.ActivationFunctionType.Exp
LN = mybir.ActivationFunctionType.Ln
MULT = mybir.AluOpType.mult
ADD = mybir.AluOpType.add
DR = mybir.MatmulPerfMode.DoubleRow


def build_nc(skip_collective=False, conv_fp8=CONV_FP8):
    nc = bacc.Bacc(None, target_bir_lowering=False)
    if conv_fp8:
        xs8 = nc.dram_tensor("xs8", [NSLOTS, 128, KO * F], FP8, kind="ExternalInput")
        wt8 = nc.dram_tensor("wt8", [128, KO * C], FP8, kind="ExternalInput")
    xbf = nc.dram_tensor("xbf", [NSLOTS, 128, KO * F], BF16, kind="ExternalInput")
    if not conv_fp8:
        wtb = nc.dram_tensor("wtb", [128, KO * C], BF16, kind="ExternalInput")
    # transposed bf16 shot inputs: xbt[i, f-part, (h, c)] = x[i, c, h*128+f]
    xbt = nc.dram_tensor("xbt", [NSHOT_SLOTS, 128, 2 * C], BF16, kind="ExternalInput")
    # bias (exp'd per-channel) and the per-shot one-hot class rows, one DMA
    bsw = nc.dram_tensor(
        "bsw", [128, KO + NSHOT_SLOTS * WAY], F32, kind="ExternalInput"
    )
    out = nc.dram_tensor("out", [1, NQ_SLOTS * WAY], F32, kind="ExternalOutput")

    with tile.TileContext(nc) as tc:
        with (
            tc.tile_pool(name="singles", bufs=1) as singles,
            tc.tile_pool(name="xepool", bufs=4) as xepool,
            tc.tile_pool(name="xms", bufs=2) as xms_pool,
            tc.tile_pool(name="xmq", bufs=NQ_SLOTS) as xmq_pool,
            tc.tile_pool(name="xm2", bufs=3) as xm2_pool,
            tc.tile_pool(name="work", bufs=8) as work,
            tc.tile_pool(name="pconv", bufs=2, space="PSUM") as pconv,
            tc.tile_pool(name="pscratch", bufs=1, space="PSUM") as pscratch,
            tc.tile_pool(name="psmall", bufs=1, space="PSUM") as psmall,
            tc.tile_pool(name="dram", bufs=2, space="DRAM") as dram,
        ):
            # ---------------- constants ----------------
            warm_bf = singles.tile([128, 256], BF16)
            nc.vector.memset(warm_bf, 0.0)
            onesC_f32 = singles.tile([128, 128], F32)
            nc.vector.memset(onesC_f32, 1.0)
            onesC_bf = singles.tile([128, 128], BF16)
            nc.vector.memset(onesC_bf, 1.0)
            onesF_bf = singles.tile([128, 1], BF16)
            nc.vector.memset(onesF_bf, 1.0 / F)
            c_eps = singles.tile([128, 1], F32)
            nc.vector.memset(c_eps, 1e-30)
            c_ln10 = singles.tile([128, 1], F32)
            nc.vector.memset(c_ln10, LN10)

            # ---- early DMA group: weights + shot inputs first (they gate the
            # conv->exp->masked-sum critical chain); bias/sw afterwards ----
            bs_sb = singles.tile([128, KO + NSHOT_SLOTS * WAY], F32)
            bias_sb = bs_sb[:, 0:KO]
            sw_sb = bs_sb[:, KO:].rearrange("p (s m) -> p s m", s=NSHOT_SLOTS)
            xball = singles.tile([128, NSLOTS, KO, F], BF16)
            xbr = xbf.rearrange("s p (ko f) -> p s ko f", ko=KO)
            xbtall = singles.tile([128, NSHOT_SLOTS, 2, C], BF16)
            xbtr = xbt.rearrange("s p (h c) -> p s h c", h=2)
            if conv_fp8:
                wt_sb = singles.tile([128, KO, C], FP8)
                x8all = singles.tile([128, NSLOTS, KO, F], FP8)
                xsr = xs8.rearrange("s p (ko f) -> p s ko f", ko=KO)
                wtr = wt8.rearrange("p (ko o) -> p ko o", ko=KO)
                nc.sync.dma_start(bs_sb, bsw[:])
                nc.sync.dma_start(wt_sb[:, 0:2], wtr[:, 0:2])
                nc.sync.dma_start(x8all[:, 0:2], xsr[:, 0:2])
                nc.sync.dma_start(wt_sb[:, 2:4], wtr[:, 2:4])
                # transposed bf16 shot copies for the mask-multiply,
                # interleaved with the fp8 conv inputs to avoid exp bubbles
                nc.sync.dma_start(x8all[:, 2:3], xsr[:, 2:3])
                nc.sync.dma_start(xbtall[:, 0:2], xbtr[:, 0:2])
                nc.sync.dma_start(x8all[:, 3:4], xsr[:, 3:4])
                nc.sync.dma_start(xbtall[:, 2:4], xbtr[:, 2:4])
            else:
                wt_sb = singles.tile([128, KO, C], BF16)
                nc.sync.dma_start(wt_sb, wtb.rearrange("p (ko o) -> p ko o", ko=KO))
                nc.sync.dma_start(xball[:, 0:2], xbr[:, 0:2])
                nc.sync.dma_start(xball[:, 2:4], xbr[:, 2:4])
                nc.sync.dma_start(bs_sb, bsw[:])

            # eb[c] = exp(b[c]) and its broadcast-column matrix for the
            # eb-weighted partition reduction of the softmax denominators
            eb = singles.tile([128, KO], F32)
            nc.scalar.activation(eb, bias_sb, EXP)
            eb2_bf = singles.tile([128, KO, 1], BF16)
            nc.scalar.activation(eb2_bf[:, :, 0], bias_sb, EXP, scale=2.0)
            ebC4 = singles.tile([128, KO, 128], BF16)
            nc.vector.tensor_copy(ebC4, eb[:, :, None].to_broadcast([128, KO, 128]))
            ebsw = singles.tile([128, KO, WAY, NSHOT_SLOTS], F32)
            nc.vector.tensor_tensor(
                ebsw,
                sw_sb.rearrange("p s m -> p m s")[:, None, :, :].to_broadcast(
                    [128, KO, WAY, NSHOT_SLOTS]
                ),
                eb[:, :, None, None].to_broadcast([128, KO, WAY, NSHOT_SLOTS]),
                MULT,
            )
            saccS = singles.tile([128, NSHOT_SLOTS * KO], BF16)
            pall = singles.tile([128, NQ_SLOTS, 2, WAY], BF16)
            out_sb = singles.tile([1, NQ_SLOTS * WAY], F32)

            # score/norm PSUM: [query, f-half, 5 scores + |q|^2] for all 10
            # queries plus the shot column-sum scratch, all in one bank.
            psall = psmall.tile([128, 512], F32)
            psT = psall[:, 0 : NQ_SLOTS * 2 * (WAY + 1)].rearrange(
                "p (a h m) -> p a h m", a=NQ_SLOTS, h=2
            )
            ps_s4 = psall[:, 136 : 136 + NSHOT_SLOTS]
            ps_sn = psall[:, 144 : 144 + WAY]
            red_ps = psall[:, 160 : 160 + NSHOT_SLOTS * KO]
            sacc_ps = psall[:, 176 : 176 + NSHOT_SLOTS * KO]

            # ---------------- PE warmup (bridges the pstate ramp) ----------
            for w in range(N_WARM):
                ps_w = pscratch.tile([128, 256], F32, tag="warm", name=f"warm{w}")
                nc.tensor.matmul(ps_w, warm_bf[:, :128], warm_bf, start=True, stop=True)

            xe_tiles = [None] * NSLOTS
            xm_tiles = [None] * NQ_SLOTS
            xm2_tiles = [None] * NQ_SLOTS

            def conv_slot(i):
                """1x1 conv for slot i -> psum tile, then exp -> xe (SBUF)."""
                ps = pconv.tile([128, KO, F], F32, tag="conv", name=f"conv{i}")
                if conv_fp8:
                    for oo in range(KO):
                        for k2 in range(KO // 2):
                            nc.tensor.matmul(
                                ps[:, oo, :],
                                wt_sb[
                                    :, 2 * k2 : 2 * k2 + 2, 128 * oo : 128 * (oo + 1)
                                ],
                                x8all[:, i, 2 * k2 : 2 * k2 + 2, :],
                                start=(k2 == 0),
                                stop=(k2 == KO // 2 - 1),
                                perf_mode=DR,
                            )
                else:
                    for oo in range(KO):
                        for k in range(KO):
                            nc.tensor.matmul(
                                ps[:, oo, :],
                                wt_sb[:, k, 128 * oo : 128 * (oo + 1)],
                                xball[:, i, k, :],
                                start=(k == 0),
                                stop=(k == KO - 1),
                            )
                xe = xepool.tile([128, KO, F], BF16, tag="xe")
                xe_tiles[i] = xe
                scale = (1.0 / W_SCALE) if conv_fp8 else 1.0
                if i == 0:
                    # split the first exp so the DVE mask chain starts half an
                    # activation earlier
                    nc.scalar.activation(xe[:, 0:2], ps[:, 0:2], EXP, scale=scale)
                    nc.scalar.activation(xe[:, 2:4], ps[:, 2:4], EXP, scale=scale)
                else:
                    nc.scalar.activation(xe, ps, EXP, scale=scale)
                return xe

            def norm_mms(j):
                # |q|^2 per position via transposed matmuls into psT col 5
                xm2 = xm2_tiles[j]
                for h in range(2):
                    for k in range(KO):
                        nc.tensor.matmul(
                            psT[:, j, h, WAY : WAY + 1],
                            xm2[:, k, 128 * h : 128 * (h + 1)],
                            eb2_bf[:, k, :],
                            start=(k == 0),
                            stop=(k == KO - 1),
                        )

            # ---------------- shot slots (transposed conv: [f, o]) --------
            for i in range(NSHOT_SLOTS):
                ps = pconv.tile([128, 2, C], F32, tag="conv", name=f"convT{i}")
                for h in range(2):
                    for k2 in range(KO // 2):
                        nc.tensor.matmul(
                            ps[:, h, :],
                            x8all[:, i, 2 * k2 : 2 * k2 + 2, 128 * h : 128 * (h + 1)],
                            wt_sb[:, 2 * k2 : 2 * k2 + 2, :],
                            start=(k2 == 0),
                            stop=(k2 == KO // 2 - 1),
                            perf_mode=DR,
                        )
                xeT = xepool.tile([128, 2, C], BF16, tag="xe")
                nc.scalar.activation(xeT, ps, EXP, scale=1.0 / W_SCALE)
                xmT = xms_pool.tile([128, 2, C], BF16, tag="xms")
                with tc.high_priority():
                    nc.vector.tensor_mul(xmT, xbtall[:, i], xeT)
                # per-channel masked sums and unweighted exp-sums as tiny
                # column matmuls (positions are on partitions)
                for k in range(KO):
                    for h in range(2):
                        nc.tensor.matmul(
                            sacc_ps[:, KO * i + k : KO * i + k + 1],
                            xeT[:, h, 128 * k : 128 * (k + 1)],
                            onesC_bf[:, 0:1],
                            start=(h == 0),
                            stop=(h == 1),
                        )
                for k in range(KO):
                    for h in range(2):
                        nc.tensor.matmul(
                            red_ps[:, KO * i + k : KO * i + k + 1],
                            xmT[:, h, 128 * k : 128 * (k + 1)],
                            onesC_bf[:, 0:1],
                            start=(h == 0),
                            stop=(h == 1),
                        )

            # ---------------- shot epilogue -> AllGather ----------------
            with tc.high_priority():
                # softmax denominators: S[i] = sum_c eb[c] * sum_f xe[c,f],
                # via one eb-weighted broadcast matmul over the column sums
                nc.vector.tensor_copy(saccS, sacc_ps)
                sv = saccS.rearrange("p (i k) -> p i k", i=NSHOT_SLOTS)
                for k in range(KO):
                    nc.tensor.matmul(
                        ps_s4,
                        ebC4[:, k, :],
                        sv[:, :, k],
                        start=(k == 0),
                        stop=(k == KO - 1),
                    )
                contrib1 = work.tile([128, KO, WAY, NSHOT_SLOTS], F32, tag="c1")
                nc.vector.tensor_tensor(
                    contrib1,
                    red_ps.rearrange("p (i k) -> p i k", i=NSHOT_SLOTS)
                    .rearrange("p i k -> p k i")[:, :, None, :]
                    .to_broadcast([128, KO, WAY, NSHOT_SLOTS]),
                    ebsw,
                    MULT,
                )
                rS4 = work.tile([128, NSHOT_SLOTS], F32, tag="rS4")
                nc.vector.reciprocal(rS4, ps_s4)
                contrib = work.tile([128, KO, WAY, NSHOT_SLOTS], F32, tag="contrib")
                nc.vector.tensor_tensor(
                    contrib,
                    contrib1,
                    rS4[:, None, None, :].to_broadcast([128, KO, WAY, NSHOT_SLOTS]),
                    MULT,
                )
                proto = work.tile([128, KO, WAY, 1], BF16, tag="proto")
                with nc.allow_low_precision(reason="bf16 prototype exchange"):
                    nc.vector.reduce_sum(proto, contrib, axis=mybir.AxisListType.X)

                ar_in = dram.tile([128, KO, WAY], BF16, tag="ar_in")
                ar_out = dram.tile([N_CORES, 128, KO, WAY], BF16, tag="ar_out")
                nc.sync.dma_start(ar_in, proto[:, :, :, 0])
            if skip_collective:
                for r in range(N_CORES):
                    nc.gpsimd.dma_start(ar_out[r], ar_in[:])
            else:
                nc.gpsimd.collective_compute(
                    "AllGather",
                    mybir.AluOpType.bypass,
                    replica_groups=[list(range(N_CORES))],
                    ins=[ar_in[:].opt()],
                    outs=[ar_out[:].opt()],
                )

            # ---- late DMA group: query inputs. Per-slot chunks keep the
            # DMA-engine FIFO shallow (the 625ns/issue HWDGE throttle paces
            # arrivals at the transfer rate), so the 10KB ar_in write slots
            # in with at most one chunk of queue delay. ----
            with tc.high_priority(offset=-50000):
                for i in range(NSHOT_SLOTS, NSLOTS):
                    if conv_fp8:
                        nc.sync.dma_start(x8all[:, i : i + 1], xsr[:, i : i + 1])
                for i in range(NSHOT_SLOTS, NSLOTS):
                    nc.sync.dma_start(xball[:, i : i + 1], xbr[:, i : i + 1])

            # ---------------- query slots ----------------
            for j in range(NQ_SLOTS):
                i = NSHOT_SLOTS + j
                xe = conv_slot(i)
                xm = xmq_pool.tile([128, KO, F], BF16, tag="xmq")
                xm_tiles[j] = xm
                nc.vector.tensor_mul(xm, xball[:, i], xe)
                xm2 = xm2_pool.tile([128, KO, F], BF16, tag="xm2")
                xm2_tiles[j] = xm2
                nc.vector.tensor_mul(xm2, xm, xm)
                if j > 0:
                    norm_mms(j - 1)
            norm_mms(NQ_SLOTS - 1)

            # per-position 10/|q| for all queries (collective-independent)
            lnq = work.tile([128, NQ_SLOTS, 2], F32, tag="lnq")
            nc.scalar.activation(lnq, psT[:, :, :, WAY], LN, bias=c_eps)
            rq = work.tile([128, NQ_SLOTS, 2], F32, tag="rq")
            nc.scalar.activation(rq, lnq, EXP, bias=c_ln10, scale=-0.5)

            # ---------------- consume AllGather ----------------
            with tc.high_priority():
                protoAll = work.tile([128, N_CORES, KO, WAY], BF16, tag="protoAll")
                nc.sync.dma_start(protoAll, ar_out.rearrange("r p k m -> p r k m"))
                protoG = work.tile([128, KO, WAY, 1], F32, tag="protoG")
                nc.vector.reduce_sum(
                    protoG,
                    protoAll.rearrange("p r k m -> p k m r"),
                    axis=mybir.AxisListType.X,
                )
                # unnormalized score weights: protoG*eb (the 10/||s|| factor
                # is folded into the logits via rqrsn below, off this path)
                s_hatU = work.tile([128, KO, WAY], BF16, tag="s_hatU")
                nc.vector.tensor_tensor(
                    s_hatU,
                    protoG[:, :, :, 0],
                    eb[:, :, None].to_broadcast([128, KO, WAY]),
                    MULT,
                )
                # ---- parallel branch: rqrsn = rq[f] * 10/||protoG||[m] ----
                protosq = work.tile([128, KO, WAY], F32, tag="protosq")
                nc.vector.tensor_mul(protosq, protoG[:, :, :, 0], protoG[:, :, :, 0])
                for k in range(KO):
                    nc.tensor.matmul(
                        ps_sn,
                        onesC_f32,
                        protosq[:, k, :],
                        start=(k == 0),
                        stop=(k == KO - 1),
                    )
                snln = work.tile([128, WAY], F32, tag="snln")
                nc.scalar.activation(snln, ps_sn, LN, bias=c_eps)
                rsnb = work.tile([128, WAY], F32, tag="rsnb")
                nc.scalar.activation(rsnb, snln, EXP, scale=-0.5)

                # ---- scores (transposed matmuls) ----
                for j in range(NQ_SLOTS):
                    xm = xm_tiles[j]
                    for h in range(2):
                        for k in range(KO):
                            nc.tensor.matmul(
                                psT[:, j, h, 0:WAY],
                                xm[:, k, 128 * h : 128 * (h + 1)],
                                s_hatU[:, k, :],
                                start=(k == 0),
                                stop=(k == KO - 1),
                            )
                rqrsn = work.tile([128, NQ_SLOTS, 2, WAY], F32, tag="rqrsn")
                nc.vector.tensor_tensor(
                    rqrsn,
                    rq[:, :, :, None].to_broadcast([128, NQ_SLOTS, 2, WAY]),
                    rsnb[:, None, None, :].to_broadcast([128, NQ_SLOTS, 2, WAY]),
                    MULT,
                )
                # ---------------- batched class softmax + spatial mean -----
                L = work.tile([128, NQ_SLOTS, 2, WAY], BF16, tag="L")
                nc.vector.tensor_tensor(L, psT[:, :, :, 0:WAY], rqrsn, MULT)
                E = work.tile([128, NQ_SLOTS, 2, WAY], BF16, tag="E")
                nc.scalar.activation(E, L, EXP)
                D = work.tile([128, NQ_SLOTS, 2, 1], F32, tag="D")
                nc.vector.reduce_sum(D, E, axis=mybir.AxisListType.X)
                R = work.tile([128, NQ_SLOTS, 2, 1], F32, tag="R")
                nc.vector.reciprocal(R, D)
                nc.vector.tensor_tensor(
                    pall, E, R.to_broadcast([128, NQ_SLOTS, 2, WAY]), MULT
                )
                psO = pscratch.tile([128, 256], F32, tag="warm", name="psO")
                for h in range(2):
                    nc.tensor.matmul(
                        psO[:1, : NQ_SLOTS * WAY],
                        onesF_bf,
                        pall[:, :, h, :],
                        start=(h == 0),
                        stop=(h == 1),
                    )
                nc.vector.tensor_copy(out_sb, psO[:1, : NQ_SLOTS * WAY])
                nc.sync.dma_start(out[:], out_sb[0:1, :])

    nc.finalize()
    return nc


_NC_CACHE = {}


def _get_nc():
    if "nc" not in _NC_CACHE:
        _NC_CACHE["nc"] = build_nc()
    return _NC_CACHE["nc"]


def _assignments():
    """Per-core (shot global ids, query global ids)."""
    shots = [20 * c + j for c in range(WAY) for j in range(SHOT)]
    queries = [20 * c + SHOT + j for c in range(WAY) for j in range(15)]
    so = np.cumsum([0] + SHOTS_PER_CORE)
    qo = np.cumsum([0] + QUERIES_PER_CORE)
    return [
        (shots[so[k] : so[k + 1]], queries[qo[k] : qo[k + 1]]) for k in range(N_CORES)
    ]


def _pack_slots(x_np, dtype):
    """[n, C, F] -> [n, 128, KO*F] p-major layout (1KB+ contiguous runs)."""
    n = x_np.shape[0]
    v = x_np.reshape(n, KO, 128, F).transpose(0, 2, 1, 3).reshape(n, 128, KO * F)
    return np.ascontiguousarray(v).astype(dtype)


def _pack_wt(W, dtype, scale=1.0):
    w = np.ascontiguousarray(W.T * scale)
    return np.ascontiguousarray(
        w.reshape(KO, 128, C).transpose(1, 0, 2).reshape(128, KO * C)
    ).astype(dtype)


def _make_in_maps(x, W, b):
    assign = _assignments()
    in_maps = []
    for k in range(N_CORES):
        s_list, q_list = assign[k]
        xs_core = np.zeros((NSLOTS, C, F), dtype=np.float32)
        xs_core[: len(s_list)] = x[s_list]
        xs_core[NSHOT_SLOTS : NSHOT_SLOTS + len(q_list)] = x[q_list]
        sw_core = np.zeros((NSHOT_SLOTS, WAY), dtype=np.float32)
        for slot, g in enumerate(s_list):
            sw_core[slot, g // 20] = 1.0
        bias_p = np.ascontiguousarray(b.reshape(KO, 128).T)  # [128, KO]
        bsw = np.concatenate(
            [
                bias_p,
                np.broadcast_to(
                    sw_core.reshape(1, NSHOT_SLOTS * WAY), (128, NSHOT_SLOTS * WAY)
                ),
            ],
            axis=1,
        ).astype(np.float32)
        # transposed shot slots: xbt[i, p, h*C + c] = x[i, c, h*128 + p]
        xbt = np.ascontiguousarray(
            xs_core[:NSHOT_SLOTS]
            .reshape(NSHOT_SLOTS, C, 2, 128)
            .transpose(0, 3, 2, 1)
            .reshape(NSHOT_SLOTS, 128, 2 * C)
        ).astype(ml_dtypes.bfloat16)
        m = {
            "xbf": _pack_slots(xs_core, ml_dtypes.bfloat16),
            "xbt": xbt,
            "bsw": np.ascontiguousarray(bsw),
        }
        if CONV_FP8:
            m["xs8"] = _pack_slots(xs_core, ml_dtypes.float8_e4m3)
            m["wt8"] = _pack_wt(W, ml_dtypes.float8_e4m3, W_SCALE)
        else:
            m["wtb"] = _pack_wt(W, ml_dtypes.bfloat16)
        in_maps.append(m)
    return in_maps


def kernel(x, W, b):
    x = np.asarray(x, dtype=np.float32).reshape(100, C, F)
    W = np.asarray(W, dtype=np.float32)
    b = np.asarray(b, dtype=np.float32)

    nc = _get_nc()
    in_maps = _make_in_maps(x, W, b)
    res = run_bass_kernel_spmd(nc, in_maps, core_ids=list(range(N_CORES)))

    assign = _assignments()
    final = np.zeros((75, WAY), dtype=np.float32)
    for k in range(N_CORES):
        out_core = np.asarray(res.results[k]["out"], dtype=np.float32).reshape(
            NQ_SLOTS, WAY
        )
        _, q_list = assign[k]
        for slot, g in enumerate(q_list):
            c, j = divmod(g, 20)
            final[15 * c + (j - SHOT)] = out_core[slot]
    return final
